# revision 17
# baseline (speedup 1.0000x reference)
"""Trainium2 Bass kernel for nn_CellDecoder (span-pool + ffnn + biaffine pairs).

Strategy: head_idx/tail_idx reference only E=256 entities, so the full set of
pair logits is exactly the E x E x OUT biaffine table per batch. The device
computes that table; host assembly indexes it per pair (the same fancy-index
assembly step the previous version already performed on the gathered output).

Sharding: the table build for batch b is split 4 ways over its cores by
(e1-half, e2-half) — core = b*4 + g1*2 + eh computes slab[g1*128:(g1+1)*128,
eh*128:(eh+1)*128, :]. Each core runs half-width (N=128) head chains for its
e1-half and tail chains for its e2-half. No inter-core communication.

Perf notes:
- The label-embedding half of the ffnn1 contraction is folded host-side into
  a [5, H1] table contracted via a one-hot tile, halving Wh1/Wt1 DMA and
  removing 30 matmuls per side.
- All matmul operands are bf16 (f32 PSUM accumulation); biases and outputs
  stay f32.
- No GPSIMD work at all: no ucode library load, no gather tail.
- DMA issue order matches compute order: pooling operands first, then
  weights in chain order, so the PE chases the stream.
"""

import os

os.environ.setdefault("JAX_PLATFORMS", "axon,cpu")

import numpy as np
import ml_dtypes

import concourse.bass as bass
import concourse.tile as tile
from concourse import bacc, mybir
from concourse.bass_utils import run_bass_kernel_spmd

dt = mybir.dt

B, T, D, E, P = 2, 512, 768, 256, 65536
MLP = 2 * D  # 1536
H1, H2 = MLP // 2, MLP // 4  # 768, 384
NL = 5
OUT = 2
N_CORES = 8
EH = E // 2  # 128, per-core entity half

KT_T = T // 128  # 4
MT_D = D // 128  # 6
KT_D = D // 128  # 6 (ffnn1 pooled contraction tiles)
KT_F1 = KT_D + 1  # 7 (+ one-hot label tile)
MT_H1 = H1 // 128  # 6
KT_H1 = H1 // 128  # 6
MT_H2 = H2 // 128  # 3
KT_H2 = H2 // 128  # 3

bf16 = ml_dtypes.bfloat16

_cache: dict = {}


def _build():
    if "nc" in _cache:
        return _cache["nc"]

    nc = bacc.Bacc("TRN2", target_bir_lowering=False, debug=False, num_devices=N_CORES)
    f32, bf = dt.float32, dt.bfloat16

    d_hs_a = nc.dram_tensor("hs_a", [128, 2 * D], bf, kind="ExternalInput")
    d_hs_b = nc.dram_tensor("hs_b", [128, 2 * D], bf, kind="ExternalInput")
    d_mask = nc.dram_tensor("mask", [128, KT_T * E], bf, kind="ExternalInput")
    d_oh = nc.dram_tensor("oh", [128, E], bf, kind="ExternalInput")
    d_w1h_a = nc.dram_tensor("w1h_a", [128, 4 * H1], bf, kind="ExternalInput")
    d_w1h_b = nc.dram_tensor("w1h_b", [128, 3 * H1], bf, kind="ExternalInput")
    d_w1t_a = nc.dram_tensor("w1t_a", [128, 4 * H1], bf, kind="ExternalInput")
    d_w1t_b = nc.dram_tensor("w1t_b", [128, 3 * H1], bf, kind="ExternalInput")
    d_w2h = nc.dram_tensor("w2h", [128, KT_H1 * H2], bf, kind="ExternalInput")
    d_w2t = nc.dram_tensor("w2t", [128, KT_H1 * H2], bf, kind="ExternalInput")
    d_wb0 = nc.dram_tensor("wb0", [128, KT_H2 * H2], bf, kind="ExternalInput")
    d_wb1 = nc.dram_tensor("wb1", [128, KT_H2 * H2], bf, kind="ExternalInput")
    d_wlin = nc.dram_tensor("wlin", [128, 2 * KT_H2 * OUT], bf, kind="ExternalInput")
    d_bh1 = nc.dram_tensor("bh1t", [128, MT_H1], f32, kind="ExternalInput")
    d_bt1 = nc.dram_tensor("bt1t", [128, MT_H1], f32, kind="ExternalInput")
    d_bh2 = nc.dram_tensor("bh2t", [128, MT_H2], f32, kind="ExternalInput")
    d_bt2 = nc.dram_tensor("bt2t", [128, MT_H2], f32, kind="ExternalInput")
    d_blin = nc.dram_tensor("blin", [1, OUT], f32, kind="ExternalInput")
    d_ones = nc.dram_tensor("ones", [1, EH], bf, kind="ExternalInput")
    d_out = nc.dram_tensor("piece", [128, OUT * EH], bf, kind="ExternalOutput")

    with tile.TileContext(nc) as tc:
        with (
            tc.tile_pool(name="w", bufs=1) as w,
            tc.tile_pool(name="act", bufs=1) as act,
            tc.tile_pool(name="ps", bufs=4, space="PSUM") as ps,
            tc.tile_pool(name="psw", bufs=2, space="PSUM") as psw,
            tc.tile_pool(name="ps1", bufs=2, space="PSUM") as ps1,
        ):
            def load(name, dram, shape, dtype=bf, engine=None):
                tl = w.tile(shape, dtype, tag=name, name=name)
                src = dram.ap()
                if len(shape) == 3:
                    src = src.rearrange("p (kt n) -> p kt n", kt=shape[1])
                (engine or nc.sync).dma_start(tl[:], src)
                return tl

            # smalls on scalar (issued up front, tiny); all bulk on the sync
            # ring in strict use order so arrival order == compute order
            oh = load("oh", d_oh, [128, E], bf, nc.scalar)
            b1 = {
                "h": load("b1h", d_bh1, [128, MT_H1], f32, nc.scalar),
                "t": load("b1t", d_bt1, [128, MT_H1], f32, nc.scalar),
            }
            b2 = {
                "h": load("b2h", d_bh2, [128, MT_H2], f32, nc.scalar),
                "t": load("b2t", d_bt2, [128, MT_H2], f32, nc.scalar),
            }
            blin = load("blin", d_blin, [1, OUT], f32, nc.scalar)
            wlin = load("wlin", d_wlin, [128, 2 * KT_H2, OUT], bf, nc.scalar)
            ones = load("ones", d_ones, [1, EH], bf, nc.scalar)

            maskt = load("mask", d_mask, [128, KT_T, E], bf)
            hs_parts = [
                load("hs_a", d_hs_a, [128, 2, D], bf),
                load("hs_b", d_hs_b, [128, 2, D], bf),
            ]
            w1 = {
                "h": [
                    load("w1h_a", d_w1h_a, [128, 4, H1], bf),
                    load("w1h_b", d_w1h_b, [128, 3, H1], bf),
                ]
            }
            w2 = {"h": load("w2h", d_w2h, [128, KT_H1, H2], bf)}
            w1["t"] = [
                load("w1t_a", d_w1t_a, [128, 4, H1], bf),
                load("w1t_b", d_w1t_b, [128, 3, H1], bf),
            ]
            w2["t"] = load("w2t", d_w2t, [128, KT_H1, H2], bf)
            wb = [
                load("wb0", d_wb0, [128, KT_H2, H2], bf),
                load("wb1", d_wb1, [128, KT_H2, H2], bf),
            ]

            # ---- pooling: entT = [pooledT ; onehot5] [128, KT_F1, E], where
            # cols 0:EH are the head-half entities and EH:E the tail-half ----
            entT = act.tile([128, KT_F1, E], bf, tag="entT")
            for mt in range(MT_D):
                p = psw.tile([128, E], f32, tag="mmw")
                for kt in range(KT_T):
                    nc.tensor.matmul(
                        p[:],
                        hs_parts[kt // 2][:, kt % 2, mt * 128 : (mt + 1) * 128],
                        maskt[:, kt, :],
                        start=(kt == 0),
                        stop=(kt == KT_T - 1),
                    )
                nc.vector.tensor_copy(entT[:, mt, :], p[:])
            nc.vector.tensor_copy(entT[:, KT_D, :], oh[:])

            # ---- ffnn chains (both sides, half-width) ----
            h2T = {}

            def ffnn(side):
                h1T = act.tile([128, KT_H1, EH], bf, tag=f"h1T{side}", name=f"h1T{side}")
                for mt in range(MT_H1):
                    p = ps.tile([128, EH], f32, tag="mm")
                    for kt in range(KT_F1):
                        w1p = w1[side][0] if kt < 4 else w1[side][1]
                        sl = slice(0, EH) if side == "h" else slice(EH, E)
                        nc.tensor.matmul(
                            p[:],
                            w1p[:, kt if kt < 4 else kt - 4, mt * 128 : (mt + 1) * 128],
                            entT[:, kt, sl],
                            start=(kt == 0),
                            stop=(kt == KT_F1 - 1),
                        )
                    nc.scalar.activation(
                        h1T[:, mt, :],
                        p[:],
                        mybir.ActivationFunctionType.Relu,
                        bias=b1[side][:, mt : mt + 1],
                    )
                h2T[side] = act.tile(
                    [128, KT_H2, EH], bf, tag=f"h2T{side}", name=f"h2T{side}"
                )
                for mt in range(MT_H2):
                    p = ps.tile([128, EH], f32, tag="mm")
                    for kt in range(KT_H1):
                        nc.tensor.matmul(
                            p[:],
                            w2[side][:, kt, mt * 128 : (mt + 1) * 128],
                            h1T[:, kt, :],
                            start=(kt == 0),
                            stop=(kt == KT_H1 - 1),
                        )
                    nc.scalar.activation(
                        h2T[side][:, mt, :],
                        p[:],
                        mybir.ActivationFunctionType.Relu,
                        bias=b2[side][:, mt : mt + 1],
                    )

            ffnn("h")
            linh = []
            for o in range(OUT):
                lh = act.tile([1, EH], bf, tag=f"linh{o}", name=f"linh{o}")
                p = ps1.tile([1, EH], f32, tag="lin")
                for kt in range(KT_H2):
                    nc.tensor.matmul(
                        p[:],
                        wlin[:, kt, o : o + 1],
                        h2T["h"][:, kt, :],
                        start=(kt == 0),
                        stop=(kt == KT_H2 - 1),
                    )
                nc.vector.tensor_copy(lh[:], p[:])
                linh.append(lh)
            ffnn("t")
            lint = []
            for o in range(OUT):
                lt = act.tile([1, EH], bf, tag=f"lint{o}", name=f"lint{o}")
                p = ps1.tile([1, EH], f32, tag="lin")
                for kt in range(KT_H2):
                    nc.tensor.matmul(
                        p[:],
                        wlin[:, KT_H2 + kt, o : o + 1],
                        h2T["t"][:, kt, :],
                        start=(kt == 0),
                        stop=(kt == KT_H2 - 1),
                    )
                nc.scalar.activation(
                    lt[:],
                    p[:],
                    mybir.ActivationFunctionType.Identity,
                    bias=blin[:, o : o + 1],
                )
                lint.append(lt)
            nT = []
            for o in range(OUT):
                nTo = act.tile([128, KT_H2, EH], bf, tag=f"nT{o}", name=f"nT{o}")
                for mt in range(MT_H2):
                    p = ps.tile([128, EH], f32, tag="mm")
                    for kt in range(KT_H2):
                        nc.tensor.matmul(
                            p[:],
                            wb[o][:, kt, mt * 128 : (mt + 1) * 128],
                            h2T["h"][:, kt, :],
                            start=(kt == 0),
                            stop=(kt == KT_H2 - 1),
                        )
                    nc.vector.tensor_copy(nTo[:, mt, :], p[:])
                nT.append(nTo)

            # ---- slab piece [128(e1), OUT, EH(e2)] bf16 ----
            out = act.tile([128, OUT, EH], bf, tag="piece")
            d_out3 = d_out.ap().rearrange("p (o e) -> p o e", o=OUT)
            for o in range(OUT):
                p = ps.tile([128, EH], f32, tag="mm")
                for kt in range(KT_H2):
                    nc.tensor.matmul(
                        p[:],
                        nT[o][:, kt, :],
                        h2T["t"][:, kt, :],
                        start=(kt == 0),
                        stop=False,
                    )
                nc.tensor.matmul(p[:], linh[o][:], ones[:], start=False, stop=False)
                nc.tensor.matmul(p[:], ones[:], lint[o][:], start=False, stop=True)
                nc.vector.tensor_copy(out[:, o, :], p[:])
                nc.sync.dma_start(d_out3[:, o, :], out[:, o, :])

    nc.compile()
    _cache["nc"] = nc
    return nc


def _pack(w_arr, kt):
    """[kt*128, n] row-major -> [128, kt*n] partition-packed, bf16."""
    n = w_arr.shape[1]
    return np.ascontiguousarray(
        w_arr.reshape(kt, 128, n).transpose(1, 0, 2).reshape(128, kt * n)
    ).astype(bf16)


def _prep_host(inputs):
    hs = np.asarray(inputs["hidden_states"], dtype=np.float32)
    start = np.asarray(inputs["entity_start"]).astype(np.int64)
    end = np.asarray(inputs["entity_end"]).astype(np.int64)
    label = np.asarray(inputs["entity_label"]).astype(np.int64)

    t = np.arange(T)
    mask = (
        (t[None, None, :] >= start[:, :, None]) & (t[None, None, :] < end[:, :, None])
    ).astype(np.float32)  # [B,E,T]
    counts = np.maximum(mask.sum(-1, keepdims=True), 1.0)
    masknT = (mask / counts).transpose(0, 2, 1)  # [B,T,E]

    def f32(x):
        return np.ascontiguousarray(np.asarray(x, dtype=np.float32))

    emb = f32(inputs["entity_emb_w"])
    Wh1, Wt1 = f32(inputs["Wh1"]), f32(inputs["Wt1"])
    # fold label-embedding half of the ffnn1 contraction into [5, H1] tables
    C1h = emb @ Wh1[D:]  # [5, H1]
    C1t = emb @ Wt1[D:]

    def w1_packed(W1, C1):
        m = np.zeros((128, KT_F1, H1), np.float32)
        m[:, :KT_D, :] = W1[:D].reshape(KT_D, 128, H1).transpose(1, 0, 2)
        m[:NL, KT_D, :] = C1
        a = np.ascontiguousarray(m[:, :4, :].reshape(128, 4 * H1)).astype(bf16)
        b = np.ascontiguousarray(m[:, 4:, :].reshape(128, 3 * H1)).astype(bf16)
        return a, b

    w1h_a, w1h_b = w1_packed(Wh1, C1h)
    w1t_a, w1t_b = w1_packed(Wt1, C1t)
    w_bil = f32(inputs["W_bil"])
    shared = {
        "w1h_a": w1h_a,
        "w1h_b": w1h_b,
        "w1t_a": w1t_a,
        "w1t_b": w1t_b,
        "w2h": _pack(f32(inputs["Wh2"]), KT_H1),
        "w2t": _pack(f32(inputs["Wt2"]), KT_H1),
        "wb0": _pack(w_bil[0], KT_H2),
        "wb1": _pack(w_bil[1], KT_H2),
        "wlin": _pack(f32(inputs["W_lin"]), 2 * KT_H2),
        "blin": f32(inputs["b_lin"]).reshape(1, OUT),
        "ones": np.ones((1, EH), bf16),
        "bh1t": np.ascontiguousarray(f32(inputs["bh1"]).reshape(MT_H1, 128).T),
        "bt1t": np.ascontiguousarray(f32(inputs["bt1"]).reshape(MT_H1, 128).T),
        "bh2t": np.ascontiguousarray(f32(inputs["bh2"]).reshape(MT_H2, 128).T),
        "bt2t": np.ascontiguousarray(f32(inputs["bt2"]).reshape(MT_H2, 128).T),
    }

    oh5 = np.zeros((B, 128, E), np.float32)
    for b in range(B):
        oh5[b, label[b], np.arange(E)] = 1.0

    in_maps = []
    for i in range(N_CORES):
        b, rest = divmod(i, 4)
        g1, eh = divmod(rest, 2)
        m = dict(shared)
        hs_p = _pack(hs[b], KT_T).reshape(128, KT_T, D)
        m["hs_a"] = np.ascontiguousarray(hs_p[:, :2, :].reshape(128, 2 * D))
        m["hs_b"] = np.ascontiguousarray(hs_p[:, 2:, :].reshape(128, 2 * D))
        m["mask"] = _pack(
            np.concatenate(
                [
                    masknT[b][:, g1 * EH : (g1 + 1) * EH],
                    masknT[b][:, eh * EH : (eh + 1) * EH],
                ],
                axis=1,
            ),
            KT_T,
        )
        m["oh"] = np.ascontiguousarray(
            np.concatenate(
                [
                    oh5[b][:, g1 * EH : (g1 + 1) * EH],
                    oh5[b][:, eh * EH : (eh + 1) * EH],
                ],
                axis=1,
            )
        ).astype(bf16)
        in_maps.append(m)
    return in_maps


def kernel(**inputs) -> np.ndarray:
    in_maps = _prep_host(inputs)
    nc = _build()
    res = run_bass_kernel_spmd(nc, in_maps, list(range(N_CORES)))

    head_idx = np.asarray(inputs["head_idx"]).astype(np.int64)
    tail_idx = np.asarray(inputs["tail_idx"]).astype(np.int64)

    out = np.zeros((B, P, OUT), np.float32)
    for b in range(B):
        slab = np.empty((E, E, OUT), np.float32)
        for rest in range(4):
            g1, eh = divmod(rest, 2)
            piece = (
                np.asarray(res.results[4 * b + rest]["piece"])
                .astype(np.float32)
                .reshape(128, OUT, EH)
            )
            slab[g1 * EH : (g1 + 1) * EH, eh * EH : (eh + 1) * EH, :] = (
                piece.transpose(0, 2, 1)
            )
        out[b] = slab[head_idx[b], tail_idx[b], :]
    return out


# revision 18
# speedup vs baseline: 1.0476x; 1.0476x over previous
"""Trainium2 Bass kernel for nn_CellDecoder (span-pool + ffnn + biaffine pairs).

Strategy: head_idx/tail_idx reference only E=256 entities, so the full set of
pair logits is exactly the E x E x OUT biaffine table per batch. The device
computes that table; host assembly indexes it per pair (the same fancy-index
assembly step the previous version already performed on the gathered output).

Sharding: the table build for batch b is split 4 ways over its cores by
(e1-half, e2-half) — core = b*4 + g1*2 + eh computes slab[g1*128:(g1+1)*128,
eh*128:(eh+1)*128, :]. Each core runs half-width (N=128) head chains for its
e1-half and tail chains for its e2-half. No inter-core communication.

Perf notes:
- The label-embedding half of the ffnn1 contraction is folded host-side into
  a [5, H1] table contracted via a one-hot tile, halving Wh1/Wt1 DMA and
  removing 30 matmuls per side.
- All matmul operands are bf16 (f32 PSUM accumulation); biases and outputs
  stay f32.
- No GPSIMD work at all: no ucode library load, no gather tail.
- DMA issue order matches compute order: pooling operands first, then
  weights in chain order, so the PE chases the stream.
"""

import os

os.environ.setdefault("JAX_PLATFORMS", "axon,cpu")

import numpy as np
import ml_dtypes

import concourse.bass as bass
import concourse.tile as tile
from concourse import bacc, mybir
from concourse.bass_utils import run_bass_kernel_spmd

dt = mybir.dt

B, T, D, E, P = 2, 512, 768, 256, 65536
MLP = 2 * D  # 1536
H1, H2 = MLP // 2, MLP // 4  # 768, 384
NL = 5
OUT = 2
N_CORES = 8
EH = E // 2  # 128, per-core entity half

KT_T = T // 128  # 4
MT_D = D // 128  # 6
KT_D = D // 128  # 6 (ffnn1 pooled contraction tiles)
KT_F1 = KT_D + 1  # 7 (+ one-hot label tile)
MT_H1 = H1 // 128  # 6
KT_H1 = H1 // 128  # 6
MT_H2 = H2 // 128  # 3
KT_H2 = H2 // 128  # 3

bf16 = ml_dtypes.bfloat16

_cache: dict = {}


def _build():
    if "nc" in _cache:
        return _cache["nc"]

    nc = bacc.Bacc("TRN2", target_bir_lowering=False, debug=False, num_devices=N_CORES)
    f32, bf = dt.float32, dt.bfloat16

    d_hs_a = nc.dram_tensor("hs_a", [128, 2 * D], bf, kind="ExternalInput")
    d_hs_b = nc.dram_tensor("hs_b", [128, 2 * D], bf, kind="ExternalInput")
    d_mask = nc.dram_tensor("mask", [128, KT_T * E], bf, kind="ExternalInput")
    d_oh = nc.dram_tensor("oh", [128, E], bf, kind="ExternalInput")
    d_w1h_a = nc.dram_tensor("w1h_a", [128, 4 * H1], bf, kind="ExternalInput")
    d_w1h_b = nc.dram_tensor("w1h_b", [128, 3 * H1], bf, kind="ExternalInput")
    d_w1t_a = nc.dram_tensor("w1t_a", [128, 4 * H1], bf, kind="ExternalInput")
    d_w1t_b = nc.dram_tensor("w1t_b", [128, 3 * H1], bf, kind="ExternalInput")
    d_w2h = nc.dram_tensor("w2h", [128, KT_H1 * H2], bf, kind="ExternalInput")
    d_w2t = nc.dram_tensor("w2t", [128, KT_H1 * H2], bf, kind="ExternalInput")
    d_wb0 = nc.dram_tensor("wb0", [128, KT_H2 * H2], bf, kind="ExternalInput")
    d_wb1 = nc.dram_tensor("wb1", [128, KT_H2 * H2], bf, kind="ExternalInput")
    d_wlin = nc.dram_tensor("wlin", [128, 2 * KT_H2 * OUT], bf, kind="ExternalInput")
    d_bh1 = nc.dram_tensor("bh1t", [128, MT_H1], f32, kind="ExternalInput")
    d_bt1 = nc.dram_tensor("bt1t", [128, MT_H1], f32, kind="ExternalInput")
    d_bh2 = nc.dram_tensor("bh2t", [128, MT_H2], f32, kind="ExternalInput")
    d_bt2 = nc.dram_tensor("bt2t", [128, MT_H2], f32, kind="ExternalInput")
    d_blin = nc.dram_tensor("blin", [1, OUT], f32, kind="ExternalInput")
    d_ones = nc.dram_tensor("ones", [1, EH], bf, kind="ExternalInput")
    d_out = nc.dram_tensor("piece", [128, OUT * EH], bf, kind="ExternalOutput")

    with tile.TileContext(nc) as tc:
        with (
            tc.tile_pool(name="w", bufs=1) as w,
            tc.tile_pool(name="act", bufs=1) as act,
            tc.tile_pool(name="ps", bufs=6, space="PSUM") as ps,
            tc.tile_pool(name="ps1", bufs=2, space="PSUM") as ps1,
        ):
            def load(name, dram, shape, dtype=bf, engine=None):
                tl = w.tile(shape, dtype, tag=name, name=name)
                src = dram.ap()
                if len(shape) == 3:
                    src = src.rearrange("p (kt n) -> p kt n", kt=shape[1])
                (engine or nc.sync).dma_start(tl[:], src)
                return tl

            # smalls on scalar (issued up front, tiny); all bulk on the sync
            # ring in strict use order so arrival order == compute order
            oh = load("oh", d_oh, [128, E], bf, nc.scalar)
            b1 = {
                "h": load("b1h", d_bh1, [128, MT_H1], f32, nc.scalar),
                "t": load("b1t", d_bt1, [128, MT_H1], f32, nc.scalar),
            }
            b2 = {
                "h": load("b2h", d_bh2, [128, MT_H2], f32, nc.scalar),
                "t": load("b2t", d_bt2, [128, MT_H2], f32, nc.scalar),
            }
            blin = load("blin", d_blin, [1, OUT], f32, nc.scalar)
            wlin = load("wlin", d_wlin, [128, 2 * KT_H2, OUT], bf, nc.scalar)
            ones = load("ones", d_ones, [1, EH], bf, nc.scalar)

            maskt = load("mask", d_mask, [128, KT_T, E], bf)
            hs_parts = [
                load("hs_a", d_hs_a, [128, 2, D], bf),
                load("hs_b", d_hs_b, [128, 2, D], bf),
            ]
            w1 = {
                "h": [
                    load("w1h_a", d_w1h_a, [128, 4, H1], bf),
                    load("w1h_b", d_w1h_b, [128, 3, H1], bf),
                ]
            }
            w2 = {"h": load("w2h", d_w2h, [128, KT_H1, H2], bf)}
            w1["t"] = [
                load("w1t_a", d_w1t_a, [128, 4, H1], bf),
                load("w1t_b", d_w1t_b, [128, 3, H1], bf),
            ]
            w2["t"] = load("w2t", d_w2t, [128, KT_H1, H2], bf)
            wb = [
                load("wb0", d_wb0, [128, KT_H2, H2], bf),
                load("wb1", d_wb1, [128, KT_H2, H2], bf),
            ]

            # ---- pooling: entT = [pooledT ; onehot5] [128, KT_F1, E], where
            # cols 0:EH are the head-half entities and EH:E the tail-half ----
            entT = act.tile([128, KT_F1, E], bf, tag="entT")
            for mt in range(MT_D):
                p = ps1.tile([128, E], f32, tag="lin")
                for kt in range(KT_T):
                    nc.tensor.matmul(
                        p[:],
                        hs_parts[kt // 2][:, kt % 2, mt * 128 : (mt + 1) * 128],
                        maskt[:, kt, :],
                        start=(kt == 0),
                        stop=(kt == KT_T - 1),
                    )
                nc.vector.tensor_copy(entT[:, mt, :], p[:])
            nc.vector.tensor_copy(entT[:, KT_D, :], oh[:])

            # ---- ffnn chains (both sides, half-width) ----
            h2T = {}

            def ffnn(side):
                h1T = act.tile([128, KT_H1, EH], bf, tag=f"h1T{side}", name=f"h1T{side}")
                for mt in range(MT_H1):
                    p = ps.tile([128, EH], f32, tag="mm")
                    for kt in range(KT_F1):
                        w1p = w1[side][0] if kt < 4 else w1[side][1]
                        sl = slice(0, EH) if side == "h" else slice(EH, E)
                        nc.tensor.matmul(
                            p[:],
                            w1p[:, kt if kt < 4 else kt - 4, mt * 128 : (mt + 1) * 128],
                            entT[:, kt, sl],
                            start=(kt == 0),
                            stop=(kt == KT_F1 - 1),
                        )
                    nc.scalar.activation(
                        h1T[:, mt, :],
                        p[:],
                        mybir.ActivationFunctionType.Relu,
                        bias=b1[side][:, mt : mt + 1],
                    )
                h2T[side] = act.tile(
                    [128, KT_H2, EH], bf, tag=f"h2T{side}", name=f"h2T{side}"
                )
                for mt in range(MT_H2):
                    p = ps.tile([128, EH], f32, tag="mm")
                    for kt in range(KT_H1):
                        nc.tensor.matmul(
                            p[:],
                            w2[side][:, kt, mt * 128 : (mt + 1) * 128],
                            h1T[:, kt, :],
                            start=(kt == 0),
                            stop=(kt == KT_H1 - 1),
                        )
                    nc.scalar.activation(
                        h2T[side][:, mt, :],
                        p[:],
                        mybir.ActivationFunctionType.Relu,
                        bias=b2[side][:, mt : mt + 1],
                    )

            ffnn("h")
            linh = []
            for o in range(OUT):
                lh = act.tile([1, EH], bf, tag=f"linh{o}", name=f"linh{o}")
                p = ps1.tile([1, EH], f32, tag="lin")
                for kt in range(KT_H2):
                    nc.tensor.matmul(
                        p[:],
                        wlin[:, kt, o : o + 1],
                        h2T["h"][:, kt, :],
                        start=(kt == 0),
                        stop=(kt == KT_H2 - 1),
                    )
                nc.vector.tensor_copy(lh[:], p[:])
                linh.append(lh)
            ffnn("t")
            lint = []
            for o in range(OUT):
                lt = act.tile([1, EH], bf, tag=f"lint{o}", name=f"lint{o}")
                p = ps1.tile([1, EH], f32, tag="lin")
                for kt in range(KT_H2):
                    nc.tensor.matmul(
                        p[:],
                        wlin[:, KT_H2 + kt, o : o + 1],
                        h2T["t"][:, kt, :],
                        start=(kt == 0),
                        stop=(kt == KT_H2 - 1),
                    )
                nc.scalar.activation(
                    lt[:],
                    p[:],
                    mybir.ActivationFunctionType.Identity,
                    bias=blin[:, o : o + 1],
                )
                lint.append(lt)
            nT = []
            for o in range(OUT):
                nTo = act.tile([128, KT_H2, EH], bf, tag=f"nT{o}", name=f"nT{o}")
                for mt in range(MT_H2):
                    p = ps.tile([128, EH], f32, tag="mm")
                    for kt in range(KT_H2):
                        nc.tensor.matmul(
                            p[:],
                            wb[o][:, kt, mt * 128 : (mt + 1) * 128],
                            h2T["h"][:, kt, :],
                            start=(kt == 0),
                            stop=(kt == KT_H2 - 1),
                        )
                    nc.vector.tensor_copy(nTo[:, mt, :], p[:])
                nT.append(nTo)

            # ---- slab piece [128(e1), OUT, EH(e2)] bf16 ----
            out = act.tile([128, OUT, EH], bf, tag="piece")
            d_out3 = d_out.ap().rearrange("p (o e) -> p o e", o=OUT)
            for o in range(OUT):
                p = ps.tile([128, EH], f32, tag="mm")
                for kt in range(KT_H2):
                    nc.tensor.matmul(
                        p[:],
                        nT[o][:, kt, :],
                        h2T["t"][:, kt, :],
                        start=(kt == 0),
                        stop=False,
                    )
                nc.tensor.matmul(p[:], linh[o][:], ones[:], start=False, stop=False)
                nc.tensor.matmul(p[:], ones[:], lint[o][:], start=False, stop=True)
                nc.vector.tensor_copy(out[:, o, :], p[:])
                nc.sync.dma_start(d_out3[:, o, :], out[:, o, :])

    nc.compile()
    _cache["nc"] = nc
    return nc


def _pack(w_arr, kt):
    """[kt*128, n] row-major -> [128, kt*n] partition-packed, bf16."""
    n = w_arr.shape[1]
    return np.ascontiguousarray(
        w_arr.reshape(kt, 128, n).transpose(1, 0, 2).reshape(128, kt * n)
    ).astype(bf16)


def _prep_host(inputs):
    hs = np.asarray(inputs["hidden_states"], dtype=np.float32)
    start = np.asarray(inputs["entity_start"]).astype(np.int64)
    end = np.asarray(inputs["entity_end"]).astype(np.int64)
    label = np.asarray(inputs["entity_label"]).astype(np.int64)

    t = np.arange(T)
    mask = (
        (t[None, None, :] >= start[:, :, None]) & (t[None, None, :] < end[:, :, None])
    ).astype(np.float32)  # [B,E,T]
    counts = np.maximum(mask.sum(-1, keepdims=True), 1.0)
    masknT = (mask / counts).transpose(0, 2, 1)  # [B,T,E]

    def f32(x):
        return np.ascontiguousarray(np.asarray(x, dtype=np.float32))

    emb = f32(inputs["entity_emb_w"])
    Wh1, Wt1 = f32(inputs["Wh1"]), f32(inputs["Wt1"])
    # fold label-embedding half of the ffnn1 contraction into [5, H1] tables
    C1h = emb @ Wh1[D:]  # [5, H1]
    C1t = emb @ Wt1[D:]

    def w1_packed(W1, C1):
        m = np.zeros((128, KT_F1, H1), np.float32)
        m[:, :KT_D, :] = W1[:D].reshape(KT_D, 128, H1).transpose(1, 0, 2)
        m[:NL, KT_D, :] = C1
        a = np.ascontiguousarray(m[:, :4, :].reshape(128, 4 * H1)).astype(bf16)
        b = np.ascontiguousarray(m[:, 4:, :].reshape(128, 3 * H1)).astype(bf16)
        return a, b

    w1h_a, w1h_b = w1_packed(Wh1, C1h)
    w1t_a, w1t_b = w1_packed(Wt1, C1t)
    w_bil = f32(inputs["W_bil"])
    shared = {
        "w1h_a": w1h_a,
        "w1h_b": w1h_b,
        "w1t_a": w1t_a,
        "w1t_b": w1t_b,
        "w2h": _pack(f32(inputs["Wh2"]), KT_H1),
        "w2t": _pack(f32(inputs["Wt2"]), KT_H1),
        "wb0": _pack(w_bil[0], KT_H2),
        "wb1": _pack(w_bil[1], KT_H2),
        "wlin": _pack(f32(inputs["W_lin"]), 2 * KT_H2),
        "blin": f32(inputs["b_lin"]).reshape(1, OUT),
        "ones": np.ones((1, EH), bf16),
        "bh1t": np.ascontiguousarray(f32(inputs["bh1"]).reshape(MT_H1, 128).T),
        "bt1t": np.ascontiguousarray(f32(inputs["bt1"]).reshape(MT_H1, 128).T),
        "bh2t": np.ascontiguousarray(f32(inputs["bh2"]).reshape(MT_H2, 128).T),
        "bt2t": np.ascontiguousarray(f32(inputs["bt2"]).reshape(MT_H2, 128).T),
    }

    oh5 = np.zeros((B, 128, E), np.float32)
    for b in range(B):
        oh5[b, label[b], np.arange(E)] = 1.0

    in_maps = []
    for i in range(N_CORES):
        b, rest = divmod(i, 4)
        g1, eh = divmod(rest, 2)
        m = dict(shared)
        hs_p = _pack(hs[b], KT_T).reshape(128, KT_T, D)
        m["hs_a"] = np.ascontiguousarray(hs_p[:, :2, :].reshape(128, 2 * D))
        m["hs_b"] = np.ascontiguousarray(hs_p[:, 2:, :].reshape(128, 2 * D))
        m["mask"] = _pack(
            np.concatenate(
                [
                    masknT[b][:, g1 * EH : (g1 + 1) * EH],
                    masknT[b][:, eh * EH : (eh + 1) * EH],
                ],
                axis=1,
            ),
            KT_T,
        )
        m["oh"] = np.ascontiguousarray(
            np.concatenate(
                [
                    oh5[b][:, g1 * EH : (g1 + 1) * EH],
                    oh5[b][:, eh * EH : (eh + 1) * EH],
                ],
                axis=1,
            )
        ).astype(bf16)
        in_maps.append(m)
    return in_maps


def kernel(**inputs) -> np.ndarray:
    in_maps = _prep_host(inputs)
    nc = _build()
    res = run_bass_kernel_spmd(nc, in_maps, list(range(N_CORES)))

    head_idx = np.asarray(inputs["head_idx"]).astype(np.int64)
    tail_idx = np.asarray(inputs["tail_idx"]).astype(np.int64)

    out = np.zeros((B, P, OUT), np.float32)
    for b in range(B):
        slab = np.empty((E, E, OUT), np.float32)
        for rest in range(4):
            g1, eh = divmod(rest, 2)
            piece = (
                np.asarray(res.results[4 * b + rest]["piece"])
                .astype(np.float32)
                .reshape(128, OUT, EH)
            )
            slab[g1 * EH : (g1 + 1) * EH, eh * EH : (eh + 1) * EH, :] = (
                piece.transpose(0, 2, 1)
            )
        out[b] = slab[head_idx[b], tail_idx[b], :]
    return out


# revision 19
# speedup vs baseline: 1.0539x; 1.0060x over previous
"""Trainium2 Bass kernel for nn_CellDecoder (span-pool + ffnn + biaffine pairs).

Strategy: head_idx/tail_idx reference only E=256 entities, so the full set of
pair logits is exactly the E x E x OUT biaffine table per batch. The device
computes that table; host assembly indexes it per pair (the same fancy-index
assembly step the previous version already performed on the gathered output).

Sharding: the table build for batch b is split 4 ways over its cores by
(e1-half, e2-half) — core = b*4 + g1*2 + eh computes slab[g1*128:(g1+1)*128,
eh*128:(eh+1)*128, :]. Each core runs half-width (N=128) head chains for its
e1-half and tail chains for its e2-half. No inter-core communication.

Perf notes (the kernel is input-DMA-stream-bound at ~4.9MB/core):
- The label-embedding half of the ffnn1 contraction is folded host-side into
  a [5, H1] table (shipped as 5 rows into a zeroed SBUF tile) contracted via
  a one-hot tile: halves Wh1/Wt1 DMA and removes 30 matmuls per side.
- Both halves' poolings are merged into one N=256 pass over a concatenated
  mask, and the two chains read column slices of one entT tile.
- All matmul operands are bf16 (f32 PSUM accumulation); biases stay f32 and
  the output pieces are written bf16 (host converts to f32).
- No GPSIMD work at all: no ucode library load, no gather tail.
- All bulk DMAs ride the sync ring in strict use order (smalls on scalar)
  so arrival order == compute order and the PE chases the stream; slab
  pieces DMA out per-o as soon as each is copied.
"""

import os

os.environ.setdefault("JAX_PLATFORMS", "axon,cpu")

import numpy as np
import ml_dtypes

import concourse.bass as bass
import concourse.tile as tile
from concourse import bacc, mybir
from concourse.bass_utils import run_bass_kernel_spmd

dt = mybir.dt

B, T, D, E, P = 2, 512, 768, 256, 65536
MLP = 2 * D  # 1536
H1, H2 = MLP // 2, MLP // 4  # 768, 384
NL = 5
OUT = 2
N_CORES = 8
EH = E // 2  # 128, per-core entity half

KT_T = T // 128  # 4
MT_D = D // 128  # 6
KT_D = D // 128  # 6 (ffnn1 pooled contraction tiles)
KT_F1 = KT_D + 1  # 7 (+ one-hot label tile)
MT_H1 = H1 // 128  # 6
KT_H1 = H1 // 128  # 6
MT_H2 = H2 // 128  # 3
KT_H2 = H2 // 128  # 3

bf16 = ml_dtypes.bfloat16

_cache: dict = {}


def _build():
    if "nc" in _cache:
        return _cache["nc"]

    nc = bacc.Bacc("TRN2", target_bir_lowering=False, debug=False, num_devices=N_CORES)
    f32, bf = dt.float32, dt.bfloat16

    d_hs_a = nc.dram_tensor("hs_a", [128, 2 * D], bf, kind="ExternalInput")
    d_hs_b = nc.dram_tensor("hs_b", [128, 2 * D], bf, kind="ExternalInput")
    d_mask = nc.dram_tensor("mask", [128, KT_T * E], bf, kind="ExternalInput")
    d_oh = nc.dram_tensor("oh", [128, E], bf, kind="ExternalInput")
    d_w1h_a = nc.dram_tensor("w1h_a", [128, 4 * H1], bf, kind="ExternalInput")
    d_w1h_b = nc.dram_tensor("w1h_b", [128, 3 * H1], bf, kind="ExternalInput")
    d_w1t_a = nc.dram_tensor("w1t_a", [128, 4 * H1], bf, kind="ExternalInput")
    d_w1t_b = nc.dram_tensor("w1t_b", [128, 3 * H1], bf, kind="ExternalInput")
    d_w2h = nc.dram_tensor("w2h", [128, KT_H1 * H2], bf, kind="ExternalInput")
    d_w2t = nc.dram_tensor("w2t", [128, KT_H1 * H2], bf, kind="ExternalInput")
    d_wb0 = nc.dram_tensor("wb0", [128, KT_H2 * H2], bf, kind="ExternalInput")
    d_wb1 = nc.dram_tensor("wb1", [128, KT_H2 * H2], bf, kind="ExternalInput")
    d_wlin = nc.dram_tensor("wlin", [128, 2 * KT_H2 * OUT], bf, kind="ExternalInput")
    d_bh1 = nc.dram_tensor("bh1t", [128, MT_H1], f32, kind="ExternalInput")
    d_bt1 = nc.dram_tensor("bt1t", [128, MT_H1], f32, kind="ExternalInput")
    d_bh2 = nc.dram_tensor("bh2t", [128, MT_H2], f32, kind="ExternalInput")
    d_bt2 = nc.dram_tensor("bt2t", [128, MT_H2], f32, kind="ExternalInput")
    d_blin = nc.dram_tensor("blin", [1, OUT], f32, kind="ExternalInput")
    d_ones = nc.dram_tensor("ones", [1, EH], bf, kind="ExternalInput")
    d_out = nc.dram_tensor("piece", [128, OUT * EH], bf, kind="ExternalOutput")

    with tile.TileContext(nc) as tc:
        with (
            tc.tile_pool(name="w", bufs=1) as w,
            tc.tile_pool(name="act", bufs=1) as act,
            tc.tile_pool(name="ps", bufs=6, space="PSUM") as ps,
            tc.tile_pool(name="ps1", bufs=2, space="PSUM") as ps1,
        ):
            def load(name, dram, shape, dtype=bf, engine=None):
                tl = w.tile(shape, dtype, tag=name, name=name)
                src = dram.ap()
                if len(shape) == 3:
                    src = src.rearrange("p (kt n) -> p kt n", kt=shape[1])
                (engine or nc.sync).dma_start(tl[:], src)
                return tl

            # smalls on scalar (issued up front, tiny); all bulk on the sync
            # ring in strict use order so arrival order == compute order
            oh = load("oh", d_oh, [128, E], bf, nc.scalar)
            b1 = {
                "h": load("b1h", d_bh1, [128, MT_H1], f32, nc.scalar),
                "t": load("b1t", d_bt1, [128, MT_H1], f32, nc.scalar),
            }
            b2 = {
                "h": load("b2h", d_bh2, [128, MT_H2], f32, nc.scalar),
                "t": load("b2t", d_bt2, [128, MT_H2], f32, nc.scalar),
            }
            blin = load("blin", d_blin, [1, OUT], f32, nc.scalar)
            wlin = load("wlin", d_wlin, [128, 2 * KT_H2, OUT], bf, nc.scalar)
            ones = load("ones", d_ones, [1, EH], bf, nc.scalar)

            maskt = load("mask", d_mask, [128, KT_T, E], bf)
            hs_parts = [
                load("hs_a", d_hs_a, [128, 2, D], bf),
                load("hs_b", d_hs_b, [128, 2, D], bf),
            ]
            w1 = {
                "h": [
                    load("w1h_a", d_w1h_a, [128, 4, H1], bf),
                    load("w1h_b", d_w1h_b, [128, 3, H1], bf),
                ]
            }
            w2 = {"h": load("w2h", d_w2h, [128, KT_H1, H2], bf)}
            w1["t"] = [
                load("w1t_a", d_w1t_a, [128, 4, H1], bf),
                load("w1t_b", d_w1t_b, [128, 3, H1], bf),
            ]
            w2["t"] = load("w2t", d_w2t, [128, KT_H1, H2], bf)
            wb = [
                load("wb0", d_wb0, [128, KT_H2, H2], bf),
                load("wb1", d_wb1, [128, KT_H2, H2], bf),
            ]

            # ---- pooling: entT = [pooledT ; onehot5] [128, KT_F1, E], where
            # cols 0:EH are the head-half entities and EH:E the tail-half ----
            entT = act.tile([128, KT_F1, E], bf, tag="entT")
            for mt in range(MT_D):
                p = ps1.tile([128, E], f32, tag="lin")
                for kt in range(KT_T):
                    nc.tensor.matmul(
                        p[:],
                        hs_parts[kt // 2][:, kt % 2, mt * 128 : (mt + 1) * 128],
                        maskt[:, kt, :],
                        start=(kt == 0),
                        stop=(kt == KT_T - 1),
                    )
                nc.vector.tensor_copy(entT[:, mt, :], p[:])
            nc.vector.tensor_copy(entT[:, KT_D, :], oh[:])

            # ---- ffnn chains (both sides, half-width) ----
            h2T = {}

            def ffnn(side):
                h1T = act.tile([128, KT_H1, EH], bf, tag=f"h1T{side}", name=f"h1T{side}")
                for mt in range(MT_H1):
                    p = ps.tile([128, EH], f32, tag="mm")
                    for kt in range(KT_F1):
                        w1p = w1[side][0] if kt < 4 else w1[side][1]
                        sl = slice(0, EH) if side == "h" else slice(EH, E)
                        nc.tensor.matmul(
                            p[:],
                            w1p[:, kt if kt < 4 else kt - 4, mt * 128 : (mt + 1) * 128],
                            entT[:, kt, sl],
                            start=(kt == 0),
                            stop=(kt == KT_F1 - 1),
                        )
                    nc.scalar.activation(
                        h1T[:, mt, :],
                        p[:],
                        mybir.ActivationFunctionType.Relu,
                        bias=b1[side][:, mt : mt + 1],
                    )
                h2T[side] = act.tile(
                    [128, KT_H2, EH], bf, tag=f"h2T{side}", name=f"h2T{side}"
                )
                for mt in range(MT_H2):
                    p = ps.tile([128, EH], f32, tag="mm")
                    for kt in range(KT_H1):
                        nc.tensor.matmul(
                            p[:],
                            w2[side][:, kt, mt * 128 : (mt + 1) * 128],
                            h1T[:, kt, :],
                            start=(kt == 0),
                            stop=(kt == KT_H1 - 1),
                        )
                    nc.scalar.activation(
                        h2T[side][:, mt, :],
                        p[:],
                        mybir.ActivationFunctionType.Relu,
                        bias=b2[side][:, mt : mt + 1],
                    )

            ffnn("h")
            linh = []
            for o in range(OUT):
                lh = act.tile([1, EH], bf, tag=f"linh{o}", name=f"linh{o}")
                p = ps1.tile([1, EH], f32, tag="lin")
                for kt in range(KT_H2):
                    nc.tensor.matmul(
                        p[:],
                        wlin[:, kt, o : o + 1],
                        h2T["h"][:, kt, :],
                        start=(kt == 0),
                        stop=(kt == KT_H2 - 1),
                    )
                nc.vector.tensor_copy(lh[:], p[:])
                linh.append(lh)
            ffnn("t")
            lint = []
            for o in range(OUT):
                lt = act.tile([1, EH], bf, tag=f"lint{o}", name=f"lint{o}")
                p = ps1.tile([1, EH], f32, tag="lin")
                for kt in range(KT_H2):
                    nc.tensor.matmul(
                        p[:],
                        wlin[:, KT_H2 + kt, o : o + 1],
                        h2T["t"][:, kt, :],
                        start=(kt == 0),
                        stop=(kt == KT_H2 - 1),
                    )
                nc.scalar.activation(
                    lt[:],
                    p[:],
                    mybir.ActivationFunctionType.Identity,
                    bias=blin[:, o : o + 1],
                )
                lint.append(lt)
            nT = []
            for o in range(OUT):
                nTo = act.tile([128, KT_H2, EH], bf, tag=f"nT{o}", name=f"nT{o}")
                for mt in range(MT_H2):
                    p = ps.tile([128, EH], f32, tag="mm")
                    for kt in range(KT_H2):
                        nc.tensor.matmul(
                            p[:],
                            wb[o][:, kt, mt * 128 : (mt + 1) * 128],
                            h2T["h"][:, kt, :],
                            start=(kt == 0),
                            stop=(kt == KT_H2 - 1),
                        )
                    nc.vector.tensor_copy(nTo[:, mt, :], p[:])
                nT.append(nTo)

            # ---- slab piece [128(e1), OUT, EH(e2)] bf16 ----
            out = act.tile([128, OUT, EH], bf, tag="piece")
            d_out3 = d_out.ap().rearrange("p (o e) -> p o e", o=OUT)
            for o in range(OUT):
                p = ps.tile([128, EH], f32, tag="mm")
                for kt in range(KT_H2):
                    nc.tensor.matmul(
                        p[:],
                        nT[o][:, kt, :],
                        h2T["t"][:, kt, :],
                        start=(kt == 0),
                        stop=False,
                    )
                nc.tensor.matmul(p[:], linh[o][:], ones[:], start=False, stop=False)
                nc.tensor.matmul(p[:], ones[:], lint[o][:], start=False, stop=True)
                nc.vector.tensor_copy(out[:, o, :], p[:])
                nc.sync.dma_start(d_out3[:, o, :], out[:, o, :])

    nc.compile()
    _cache["nc"] = nc
    return nc


def _pack(w_arr, kt):
    """[kt*128, n] row-major -> [128, kt*n] partition-packed, bf16."""
    n = w_arr.shape[1]
    return np.ascontiguousarray(
        w_arr.reshape(kt, 128, n).transpose(1, 0, 2).reshape(128, kt * n)
    ).astype(bf16)


def _prep_host(inputs):
    hs = np.asarray(inputs["hidden_states"], dtype=np.float32)
    start = np.asarray(inputs["entity_start"]).astype(np.int64)
    end = np.asarray(inputs["entity_end"]).astype(np.int64)
    label = np.asarray(inputs["entity_label"]).astype(np.int64)

    t = np.arange(T)
    mask = (
        (t[None, None, :] >= start[:, :, None]) & (t[None, None, :] < end[:, :, None])
    ).astype(np.float32)  # [B,E,T]
    counts = np.maximum(mask.sum(-1, keepdims=True), 1.0)
    masknT = (mask / counts).transpose(0, 2, 1)  # [B,T,E]

    def f32(x):
        return np.ascontiguousarray(np.asarray(x, dtype=np.float32))

    emb = f32(inputs["entity_emb_w"])
    Wh1, Wt1 = f32(inputs["Wh1"]), f32(inputs["Wt1"])
    # fold label-embedding half of the ffnn1 contraction into [5, H1] tables
    C1h = emb @ Wh1[D:]  # [5, H1]
    C1t = emb @ Wt1[D:]

    def w1_packed(W1, C1):
        m = np.zeros((128, KT_F1, H1), np.float32)
        m[:, :KT_D, :] = W1[:D].reshape(KT_D, 128, H1).transpose(1, 0, 2)
        m[:NL, KT_D, :] = C1
        a = np.ascontiguousarray(m[:, :4, :].reshape(128, 4 * H1)).astype(bf16)
        b = np.ascontiguousarray(m[:, 4:, :].reshape(128, 3 * H1)).astype(bf16)
        return a, b

    w1h_a, w1h_b = w1_packed(Wh1, C1h)
    w1t_a, w1t_b = w1_packed(Wt1, C1t)
    w_bil = f32(inputs["W_bil"])
    shared = {
        "w1h_a": w1h_a,
        "w1h_b": w1h_b,
        "w1t_a": w1t_a,
        "w1t_b": w1t_b,
        "w2h": _pack(f32(inputs["Wh2"]), KT_H1),
        "w2t": _pack(f32(inputs["Wt2"]), KT_H1),
        "wb0": _pack(w_bil[0], KT_H2),
        "wb1": _pack(w_bil[1], KT_H2),
        "wlin": _pack(f32(inputs["W_lin"]), 2 * KT_H2),
        "blin": f32(inputs["b_lin"]).reshape(1, OUT),
        "ones": np.ones((1, EH), bf16),
        "bh1t": np.ascontiguousarray(f32(inputs["bh1"]).reshape(MT_H1, 128).T),
        "bt1t": np.ascontiguousarray(f32(inputs["bt1"]).reshape(MT_H1, 128).T),
        "bh2t": np.ascontiguousarray(f32(inputs["bh2"]).reshape(MT_H2, 128).T),
        "bt2t": np.ascontiguousarray(f32(inputs["bt2"]).reshape(MT_H2, 128).T),
    }

    oh5 = np.zeros((B, 128, E), np.float32)
    for b in range(B):
        oh5[b, label[b], np.arange(E)] = 1.0

    in_maps = []
    for i in range(N_CORES):
        b, rest = divmod(i, 4)
        g1, eh = divmod(rest, 2)
        m = dict(shared)
        hs_p = _pack(hs[b], KT_T).reshape(128, KT_T, D)
        m["hs_a"] = np.ascontiguousarray(hs_p[:, :2, :].reshape(128, 2 * D))
        m["hs_b"] = np.ascontiguousarray(hs_p[:, 2:, :].reshape(128, 2 * D))
        m["mask"] = _pack(
            np.concatenate(
                [
                    masknT[b][:, g1 * EH : (g1 + 1) * EH],
                    masknT[b][:, eh * EH : (eh + 1) * EH],
                ],
                axis=1,
            ),
            KT_T,
        )
        m["oh"] = np.ascontiguousarray(
            np.concatenate(
                [
                    oh5[b][:, g1 * EH : (g1 + 1) * EH],
                    oh5[b][:, eh * EH : (eh + 1) * EH],
                ],
                axis=1,
            )
        ).astype(bf16)
        in_maps.append(m)
    return in_maps


def kernel(**inputs) -> np.ndarray:
    in_maps = _prep_host(inputs)
    nc = _build()
    res = run_bass_kernel_spmd(nc, in_maps, list(range(N_CORES)))

    head_idx = np.asarray(inputs["head_idx"]).astype(np.int64)
    tail_idx = np.asarray(inputs["tail_idx"]).astype(np.int64)

    out = np.zeros((B, P, OUT), np.float32)
    for b in range(B):
        slab = np.empty((E, E, OUT), np.float32)
        for rest in range(4):
            g1, eh = divmod(rest, 2)
            piece = (
                np.asarray(res.results[4 * b + rest]["piece"])
                .astype(np.float32)
                .reshape(128, OUT, EH)
            )
            slab[g1 * EH : (g1 + 1) * EH, eh * EH : (eh + 1) * EH, :] = (
                piece.transpose(0, 2, 1)
            )
        out[b] = slab[head_idx[b], tail_idx[b], :]
    return out
